# revision 47
# baseline (speedup 1.0000x reference)
"""Causal attention block (q/k/v proj + causal softmax + concat) on 8 trn2 cores.

Sharding: batch n -> core pair (2n, 2n+1); within a batch the 16 query
chunks of 256 rows are split by parity (core p owns chunks 2s+p, s=0..7).
The host hands each core a row-permuted x^T in fp16 (own chunks first,
then the other core's chunks), so all 8 cores run one uniform SPMD
program; causal-mask differences are pure input data (multiplicative
0/1 mask tiles). Attention runs in the transposed orientation
(logits^T tiles [tk=128, tq=512]), with fp16 matmul operands (fp32 PSUM
accumulation); the softmax denominator is fused into the read matmul
via a ones-column appended to v.

Block pairing: group g of query unit u processes own 128-block g at
PE rows 0:64 (j=0) and other-core block g at rows 64:128 (j=1). This
fixes which partition half each chunk's k lives in, so projections are
single full-width packed passes: [Wq|Wq] (q duplicated in both halves,
the moving operand of both j's), [Wk|Wv] for own chunks (k low, v
high), [Wv|Wk] for other chunks (k high, v low). One pass + one fused
bias-add per chunk. The j=0/j=1 halves also give the attention matmuls
their LDWEIGHTS ping-pong.

v2: the device returns the raw accumulator po [65, 512] per unit
(read^T rows 0:64, softmax denominator row 64) straight from PSUM; the
division and transpose happen on the host. A warm-up burst of dummy
matmuls at kernel start trips the PE HAM clock-gate to 2.4 GHz before
the first data-dependent matmul issues, and the x^T DMA stream is
split need-ordered across the sync and scalar HWDGE queues so the
first projection starts as soon as chunk 0+4 quadrants land.
"""

from contextlib import ExitStack

import numpy as np

import concourse.bacc as bacc
import concourse.mybir as mybir
import concourse.tile as tile
from concourse.bass_utils import run_bass_kernel_spmd

F32 = mybir.dt.float32
F16 = mybir.dt.float16
ADD = mybir.AluOpType.add
EXP = mybir.ActivationFunctionType.Exp

N, T, C, KD, VD = 4, 4096, 512, 64, 64
CH = 256          # query chunk rows
SLOTS = 8         # own 256-chunks per core
TQ = SLOTS * CH   # 2048 own query rows per core
SCALE = 1.0 / 8.0
NWARM = 8         # HAM warm-up matmuls (~3.4us cold, accumulation group)

_NC_CACHE = None
_LAST_IN_MAPS = None


def _build():
    nc = bacc.Bacc("TRN2", target_bir_lowering=False, debug=False)

    xT_d = nc.dram_tensor("xT", [C, T], F16, kind="ExternalInput").ap()
    wq_d = nc.dram_tensor("wq", [4, 128, 128], F16, kind="ExternalInput").ap()
    wkv_d = nc.dram_tensor("wkv", [2, 4, 128, 128], F16, kind="ExternalInput").ap()
    bias_d = nc.dram_tensor("bias", [128, 4], F32, kind="ExternalInput").ap()
    masks_d = nc.dram_tensor("masks", [128, 4, CH], F16, kind="ExternalInput").ap()
    ident16_d = nc.dram_tensor("ident16", [128, 128], F16, kind="ExternalInput").ap()
    # per unit u: rows 0:64 = read^T (v-dim major), row 64 = softmax denom
    out_d = nc.dram_tensor("out", [4, VD + 1, 512], F32, kind="ExternalOutput").ap()

    with tile.TileContext(nc) as tc, ExitStack() as ctx:
        const = ctx.enter_context(tc.tile_pool(name="const", bufs=1))
        data = ctx.enter_context(tc.tile_pool(name="data", bufs=1))

        # constants on the scalar (Activation) HWDGE queue — its DMA
        # channel is slow (~5x below sync's ~190 GB/s) so it carries
        # only the ~700 KB of weights/masks; the whole 4 MB x stream
        # rides the sync queue in consumption order.
        wkv_sb = [const.tile([128, 4, 128], F16, name=f"wkv{s}") for s in range(2)]
        for s in range(2):
            nc.scalar.dma_start(out=wkv_sb[s],
                                in_=wkv_d[s].rearrange("a p m -> p a m"))
        wq_sb = const.tile([128, 4, 128], F16)
        nc.scalar.dma_start(out=wq_sb, in_=wq_d.rearrange("a p m -> p a m"))
        bias_sb = const.tile([128, 4], F32)
        nc.scalar.dma_start(out=bias_sb, in_=bias_d)
        ident16_sb = const.tile([128, 128], F16)
        nc.scalar.dma_start(out=ident16_sb, in_=ident16_d)
        masks_sb = const.tile([128, 4, CH], F16)
        nc.scalar.dma_start(out=masks_sb, in_=masks_d)

        # warm-up source: zeroed by gpsimd (its queue is idle after the
        # preamble) so the dummy matmuls depend on nothing else.
        dummy_sb = const.tile([128, 512], F16)
        nc.gpsimd.memset(dummy_sb, 0.0)

        bq_col = bias_sb[:, 0:1]     # [bq; bq]
        bkv_col = bias_sb[:, 1:2]    # [bk; bv]  (own chunks)
        bvk_col = bias_sb[:, 2:3]    # [bv; bk]  (other chunks)

        # x^T tiles in consumption order on the sync queue. The t-axis
        # splits into halves at col 2048 (own chunks 0-3 | other chunks
        # 4-7); one strided descriptor fetches the same 512-col quadrant
        # of BOTH halves, so chunks 0 and 4 arrive together (the first
        # logits need both); per-queue DMA bandwidth is the same for
        # this pattern as for contiguous tiles (~190 GB/s).
        xpA = [data.tile([128, 2, 512], F16, name=f"xpA{c}") for c in range(4)]
        xpB = [data.tile([128, 2, 512], F16, name=f"xpB{c}") for c in range(4)]
        xt1 = [data.tile([128, 1024], F16, name=f"xt1_{c}") for c in range(4)]
        xt3 = [data.tile([128, 1024], F16, name=f"xt3_{c}") for c in range(4)]
        for xp, lo in ((xpA, 0), (xpB, 512)):
            for c in range(4):
                nc.sync.dma_start(
                    out=xp[c],
                    in_=xT_d[c * 128:(c + 1) * 128]
                        .rearrange("p (b q) -> p b q", b=2)[:, :, lo:lo + 512],
                )
        for xt, h in ((xt1, 1), (xt3, 3)):
            for c in range(4):
                nc.sync.dma_start(
                    out=xt[c],
                    in_=xT_d[c * 128:(c + 1) * 128, h * 1024:(h + 1) * 1024],
                )

        def x_src(k, c):
            """Moving operand [128, 512] of projection chunk k, c-chunk c."""
            if k in (0, 4):
                return xpA[c][:, k // 4, :]
            if k in (1, 5):
                return xpB[c][:, k // 4, :]
            xt = xt1 if k < 4 else xt3
            return xt[c][:, (k % 2) * 512:(k % 2) * 512 + 512]

        v_ext = data.tile([128, 32, VD + 1], F16)
        # ones column for the fused softmax denominator: (x*0)+1 via
        # tensor_scalar (memset can't cast); wkv0 is the earliest-arriving
        # fp16 tile so it serves as the dummy input.
        nc.vector.tensor_scalar(
            out=v_ext[:, :, VD:VD + 1],
            in0=wkv_sb[0][:, 0, 0:32].unsqueeze(2),
            scalar1=0.0, scalar2=1.0,
            op0=mybir.AluOpType.mult, op1=ADD,
        )

        q_sb = [data.tile([128, 512], F16, name=f"q{k}") for k in range(4)]
        # kv_sb[k]: own chunks: rows 0:64 k^T, 64:128 v^T;
        #           other chunks: rows 0:64 v^T, 64:128 k^T.
        kv_sb = [data.tile([128, 512], F16, name=f"kv{k}") for k in range(8)]

        ps_o = ctx.enter_context(tc.tile_pool(name="ps_o", bufs=2, space="PSUM"))
        ptp = ctx.enter_context(tc.tile_pool(name="ptp", bufs=6))
        finp = ctx.enter_context(tc.tile_pool(name="finp", bufs=2))

        IDENT = mybir.ActivationFunctionType.Identity

        def proj_pieces(k, ps_pr):
            """Projection chunk k as a list of sub-us emission pieces.

            All PSUM->SBUF bias-adds/copies run on DVE so the Scalar
            (ACT) queue carries nothing but the softmax exps — ACT is
            the whole-kernel pacing engine.
            """
            own = k < 4
            vrows = slice(64, 128) if own else slice(0, 64)

            def bias_add(out, in_, col):
                nc.vector.tensor_scalar(
                    out=out, in0=in_, scalar1=col, scalar2=None, op0=ADD)

            def piece_kv():
                ps = ps_pr.tile([128, 512], F32, name=f"pskv{k}", tag="ps")
                for c in range(4):
                    nc.tensor.matmul(
                        ps, wkv_sb[0 if own else 1][:, c, :],
                        x_src(k, c),
                        start=(c == 0), stop=(c == 3),
                    )
                bias_add(kv_sb[k], ps, bkv_col if own else bvk_col)

            def piece_q():
                psq = ps_pr.tile([128, 512], F32, name=f"psq{k}", tag="ps")
                for c in range(4):
                    nc.tensor.matmul(
                        psq, wq_sb[:, c, :], x_src(k, c),
                        start=(c == 0), stop=(c == 3),
                    )
                bias_add(q_sb[k], psq, bq_col)

            def piece_t():
                pvt = ps_pr.tile([128, 4, VD], F16, name=f"pvt{k}", tag="ps")
                idn = ident16_sb[vrows, vrows]
                for j in range(4):
                    nc.tensor.transpose(
                        pvt[:, j, :], kv_sb[k][vrows, j * 128:(j + 1) * 128],
                        idn)
                s0 = 4 * k if own else 16 + 4 * (k - 4)
                nc.vector.tensor_copy(v_ext[:, s0:s0 + 4, 0:VD], pvt)

            if own:
                return [piece_kv, piece_q, piece_t]
            return [piece_kv, piece_t]

        # Schraudolph fast-exp on DVE: fp16 bits of e^(x*SCALE) are
        # approximately int16(A*x + B); one tensor_scalar (mult, add)
        # writing int16 bits into the fp16 pt tile via bitcast. ~3%
        # oscillating per-element error that cancels between softmax
        # numerator and denominator (simulated end-to-end 4.5e-4 vs the
        # 2e-2 gate). Used for alternating unmasked groups of the
        # second couple, where ACT paces the kernel and DVE has slack.
        FA = 1024.0 / float(np.log(2.0)) * SCALE
        FB = 15360.0 - 44.5
        fast_st = {"on": False, "ctr": 0}

        def emit_logits(u, g, ps_l):
            """Logits + exp (+mask) for group g of unit u; returns (pt, narrow)."""
            nG = 4 * u + 4
            qrhs = q_sb[u]
            narrow = g >= nG - 2
            pl = ps_l.tile([128, 2, 512], F32, name=f"pl{u}_{g}", tag="pl")
            for j in range(2):
                half = j * KD
                kw = kv_sb[g // 4 + 4 * j][half:half + KD,
                                           (g % 4) * 128:(g % 4) * 128 + 128]
                if narrow:
                    nc.tensor.matmul(pl[:, j, 0:CH], kw,
                                     qrhs[half:half + KD, CH:512],
                                     start=True, stop=True)
                else:
                    nc.tensor.matmul(pl[:, j, :], kw,
                                     qrhs[half:half + KD, :],
                                     start=True, stop=True)
            pt = ptp.tile([128, 2, 512], F16, name="pt", tag="pt")
            masked = g >= nG - 4
            if narrow:
                nc.scalar.activation(out=pt[:, :, 0:CH], in_=pl[:, :, 0:CH],
                                     func=EXP, scale=SCALE)
            elif fast_st["on"] and not masked and fast_st["ctr"] % 2 == 0:
                fast_st["ctr"] += 1
                nc.vector.tensor_scalar(
                    out=pt[:, :, :].bitcast(mybir.dt.int16),
                    in0=pl, scalar1=FA, scalar2=FB,
                    op0=mybir.AluOpType.mult, op1=ADD)
            else:
                if fast_st["on"] and not masked:
                    fast_st["ctr"] += 1
                nc.scalar.activation(out=pt, in_=pl, func=EXP, scale=SCALE)
            if masked:
                mi = ((g - (nG - 4)) % 2) * 2
                nc.vector.tensor_mul(
                    pt[:, :, 0:CH], pt[:, :, 0:CH], masks_sb[:, mi:mi + 2, :])
            return pt, narrow

        def emit_read(u, g, pt, narrow, po):
            nG = 4 * u + 4
            for j in range(2):
                b = g if j == 0 else 16 + g
                nc.tensor.matmul(
                    po[:, CH:512] if narrow else po,
                    v_ext[:, b, :],
                    pt[:, j, 0:CH] if narrow else pt[:, j, :],
                    start=(g == 0 and j == 0),
                    stop=(g == nG - 1 and j == 1),
                )

        def finalize(u, po):
            # raw accumulator out via SBUF (DMA can't read PSUM); the
            # division by the denominator row happens on the host.
            osb = finp.tile([VD + 1, 512], F32, name="osb", tag="osb")
            nc.vector.tensor_copy(osb, po)
            nc.sync.dma_start(out=out_d[u], in_=osb)

        def emit_couple(ua, ub, ps_l, ps_pr, weave, weave_a=None, pre_a=None):
            """Interleave units ua < ub group-by-group, with the reads
            software-pipelined ONE slot behind the logits: at slot g the
            PE queue holds logits(g) plus reads(g-1), whose exps were
            issued a full slot earlier — so the PE never drains waiting
            on the current group's exp (that wait is what exposed the
            375ns isolated-matmul latency). weave_a pieces go between
            the two units' logits."""
            po_a = ps_o.tile([VD + 1, 512], F32, name=f"po{ua}", tag="po")
            po_b = ps_o.tile([VD + 1, 512], F32, name=f"po{ub}", tag="po")
            na, nb = 4 * ua + 4, 4 * ub + 4
            pend = []

            def flush():
                for u, g, pt, nar, po in pend:
                    emit_read(u, g, pt, nar, po)
                pend.clear()

            for g in range(max(na, nb)):
                if pre_a and g in pre_a:
                    ra = pre_a.pop(g)   # logits pre-emitted in prior couple
                else:
                    ra = emit_logits(ua, g, ps_l) if g < na else None
                for piece in (weave_a or {}).get(g, ()):
                    piece()
                rb = emit_logits(ub, g, ps_l) if g < nb else None
                for piece in weave.get(g, ()):
                    piece()
                flush()
                if ra is not None:
                    pend.append((ua, g, ra[0], ra[1], po_a))
                if g == na:
                    finalize(ua, po_a)
                if rb is not None:
                    pend.append((ub, g, rb[0], rb[1], po_b))
            flush()
            finalize(ub, po_b)

        with tc.tile_pool(name="ps_pr", bufs=2, space="PSUM") as ps_pr, \
             tc.tile_pool(name="ps_la", bufs=2, space="PSUM") as ps_la:
            # HAM warm-up: one PSUM accumulation group of dummy matmuls
            # (no WAW semaphores between them, so the array streams them
            # back-to-back) keeps the PE continuously busy from
            # preamble-end (~7.5us) until the first x tiles land
            # (~11us); the clock gate needs ~3.4us of uninterrupted
            # activity to open to 2.4 GHz.
            pw = ps_la.tile([128, 512], F32, name="warm", tag="pl")
            for w in range(NWARM):
                nc.tensor.matmul(pw, dummy_sb[:, 0:128], dummy_sb,
                                 start=(w == 0), stop=(w == NWARM - 1))

            pc = {k: proj_pieces(k, ps_pr) for k in range(8)}
            # own chunk k<4 -> [kv, q, t]; other -> [kv, t]
            # minimal head: the first exp (u0 g0) only needs kv0/kv4/q0;
            # everything else weaves into the couple loop.
            pc[0][0](); pc[4][0]()        # kv0, kv4
            pc[0][1]()                    # q0
            # couple (0,1): t0/t4 must land before the g=0 reads, q1
            # before u1's g=0 logits (weave_a), kv1/t1/kv5/t5 before
            # loop g=4; q2/q3 before couple 2.
            # couple-boundary overlap: u2's first two logits groups are
            # pre-emitted in couple 1's last (narrow, light) slot so
            # couple 2 starts with its exp pipeline already primed; by
            # then q2 (woven at slot 4) and kv0/kv4 are resident.
            pre2 = {}
            weave_a1 = {0: [pc[1][1]]}    # q1
            weave1 = {
                0: [pc[0][2], pc[4][1], pc[1][0]],  # t0, t4, kv1
                1: [pc[1][2], pc[5][0]],  # t1, kv5
                2: [pc[5][1]],            # t5
                4: [pc[2][1]],            # q2
                5: [pc[3][1]],            # q3
                7: [lambda: pre2.__setitem__(0, emit_logits(2, 0, ps_la)),
                    lambda: pre2.__setitem__(1, emit_logits(2, 1, ps_la))],
            }
            emit_couple(0, 1, ps_la, ps_pr, weave1, weave_a1)
            fast_st["on"] = True   # couple 2: ACT paces; offload to DVE
            # couple (2,3): kv2/t2/kv6/t6 before loop g=8; kv3/t3/kv7/t7
            # before loop g=12.
            weave2 = {
                0: [pc[2][0]],            # kv2
                1: [pc[2][2]],            # t2
                2: [pc[6][0]],            # kv6
                3: [pc[6][1]],            # t6
                8: [pc[3][0]],            # kv3
                9: [pc[3][2]],            # t3
                10: [pc[7][0]],           # kv7
                11: [pc[7][1]],           # t7
            }
            emit_couple(2, 3, ps_la, ps_pr, weave2, pre_a=pre2)

    nc.compile()
    return nc


def _get_nc():
    global _NC_CACHE
    if _NC_CACHE is None:
        _NC_CACHE = _build()
    return _NC_CACHE


def _make_masks(p):
    """Mask tiles [128, 4, 256] for parity p: [m0, mp, m1, mp].

    Group 4u+d (d in 0..3) applies masks [m0,mp] (d even) or [m1,mp]
    (d odd) to pt[:, 0:2, 0:CH]: j=0 own block is on the causal diagonal
    (triangle d%2), j=1 other block is dead (p=0) or alive (p=1) there.
    """
    i = np.arange(128)[:, None]
    j = np.arange(CH)[None, :]
    m = np.zeros((128, 4, CH), dtype=np.float16)
    m[:, 0, :] = (j >= i)             # diag tri, tk-block even
    m[:, 2, :] = (j >= 128 + i)       # diag tri, tk-block odd
    m[:, 1, :] = m[:, 3, :] = 1.0 if p == 1 else 0.0
    return m


def kernel(x, Wq, bq, Wk, bk, Wv, bv):
    x = np.asarray(x, dtype=np.float32)
    Wq = np.asarray(Wq, dtype=np.float32)
    Wk = np.asarray(Wk, dtype=np.float32)
    Wv = np.asarray(Wv, dtype=np.float32)
    bq = np.asarray(bq, dtype=np.float32)
    bk = np.asarray(bk, dtype=np.float32)
    bv = np.asarray(bv, dtype=np.float32)

    nc = _get_nc()

    ident16 = np.eye(128, dtype=np.float16)
    wq_h = np.ascontiguousarray(
        np.tile(Wq.reshape(4, 128, KD), (1, 1, 2)).astype(np.float16))
    wk4 = Wk.reshape(4, 128, KD).astype(np.float16)
    wv4 = Wv.reshape(4, 128, KD).astype(np.float16)
    wkv_h = np.ascontiguousarray(np.stack([
        np.concatenate([wk4, wv4], axis=2),         # own pack: [Wk|Wv]
        np.concatenate([wv4, wk4], axis=2),         # other pack: [Wv|Wk]
    ]))                                             # [2,4,128,128]
    bias_h = np.zeros((128, 4), dtype=np.float32)
    bias_h[:, 0] = np.tile(bq, 2)
    bias_h[:, 1] = np.concatenate([bk, bv])
    bias_h[:, 2] = np.concatenate([bv, bk])
    masks_p = [_make_masks(0), _make_masks(1)]

    in_maps = []
    for core in range(8):
        n, p = core // 2, core % 2
        perm = [2 * s + p for s in range(8)] + [2 * s + 1 - p for s in range(8)]
        xp = x[n].reshape(16, CH, C)[perm].reshape(T, C)
        in_maps.append({
            "xT": np.ascontiguousarray(xp.T.astype(np.float16)),
            "wq": wq_h, "wkv": wkv_h, "bias": bias_h,
            "masks": masks_p[p], "ident16": ident16,
        })

    global _LAST_IN_MAPS
    _LAST_IN_MAPS = in_maps
    res = run_bass_kernel_spmd(nc, in_maps, core_ids=list(range(8)))

    out = np.empty((N, T, C + VD), dtype=np.float32)
    out[:, :, :C] = x
    for core in range(8):
        n, p = core // 2, core % 2
        co = res.results[core]["out"]            # [4, 65, 512]
        rd = co[:, :VD, :] / co[:, VD:VD + 1, :]  # divide by softmax denom
        for s in range(8):
            u, h = divmod(s, 2)
            g0 = (2 * s + p) * CH
            out[n, g0:g0 + CH, C:] = rd[u, :, h * CH:(h + 1) * CH].T
    return out
